# revision 3
# baseline (speedup 1.0000x reference)
"""Trainium2 Bass kernel for masked multi-head attention with adjacency-derived
sparse masks (nn_MultiHeadAttention_4922032521398).

Reference (per batch of 32, L=512, DIM=256, 4 heads x 64):
    qkv = x @ w_qkv.T ; q,k,v per head
    score = q @ k.T / sqrt(64)
    a   = binarize(adj): 1 where adj==1 or adj>=9 else 0
    pe  = stack([a, aT, aT@a, a@aT]) + I   (per-head masks, !=0 -> keep)
    out = softmax(where(pe==0, -inf, score)) @ v ; y = out @ w_proj.T

Strategy (data-parallel over batch across 8 cores, 4 batches each):
  - Scores built transposed: S^T[k,q] so attention@V and the projection
    contract without any on-device transposes.  P^T = exp(S^T/8)*mask^T;
    scores are small (|s|<~2) so exp needs no max-subtraction, and the 0/1
    mask multiply equals -inf masking exactly.
  - Host precomputes the adjacency binarization: pure 0/1 bins go up as
    fp8 (exact) for the head-2/3 count matmuls; (a|I)/(aT|I) go up as bf16
    and are used directly as the head-0/1 masks.  No device binarize, no
    fp8 conversion, no identity-OR passes.
  - Heads 2/3: adjacency counts (aT@a / a@aT as fp8 DoubleRow matmuls on
    exact 0/1 values, fp32 PSUM accumulate => exact counts) stay in PSUM
    and fuse into the softmax as P^T=(count>=0.5)*exp(S^T/8) in one DVE
    scalar_tensor_tensor op; the +I lands via a 128-col identity matmul
    that also closes the accumulation group.
  - Heads 0/1: P^T = exp * (a|I)-mask on GPSIMD (SBUF-only op, Pool was
    idle; DVE/ACT are the contended engines).
  - Row sums via a ones-column appended to V (PV matmul row 64); applied
    as DVE reciprocal -> GPSIMD partition_broadcast -> DVE multiply (no
    PE broadcast matmul, no ACT copy).
  - x / weights in bf16 (halves DMA + SBUF; well inside the 2e-2 budget);
    attention P/V in bf16; all matmuls full-rate.
  - Elementwise stages operate on 2-bank PSUM tiles ([128,2,512]) to halve
    per-op fixed costs on ACT/DVE.
"""

import os
import sys

os.environ.setdefault("JAX_PLATFORMS", "axon,cpu")

for _p in ("/opt/trn_rl_repo",):
    if _p not in sys.path:
        sys.path.append(_p)

import numpy as np
import ml_dtypes

import concourse.bass as bass
import concourse.mybir as mybir
import concourse.tile as tile
from concourse import bacc
from concourse.bass_utils import run_bass_kernel_spmd
from concourse.masks import make_identity

B, L, DIM, NH = 32, 512, 256, 4
HD = DIM // NH  # 64
SCALE = float(np.sqrt(HD))
NCORES = 8
BPC = B // NCORES  # batches per core

F32 = mybir.dt.float32
BF16 = mybir.dt.bfloat16
FP8 = mybir.dt.float8e4
AF = mybir.ActivationFunctionType
OP = mybir.AluOpType
DR = mybir.MatmulPerfMode.DoubleRow


def build_nc():
    nc = bacc.Bacc("TRN2", target_bir_lowering=False)
    xT_d = nc.declare_dram_parameter("xT", [BPC, DIM, L], BF16, isOutput=False)
    a8_d = nc.declare_dram_parameter("a8", [BPC, L, L], FP8, isOutput=False)
    a8T_d = nc.declare_dram_parameter("a8T", [BPC, L, L], FP8, isOutput=False)
    mh0_d = nc.declare_dram_parameter("mh0", [BPC, L, L], BF16, isOutput=False)
    mh1_d = nc.declare_dram_parameter("mh1", [BPC, L, L], BF16, isOutput=False)
    wqkvT_d = nc.declare_dram_parameter("wqkvT", [DIM, 3 * DIM], BF16, isOutput=False)
    wprojT_d = nc.declare_dram_parameter("wprojT", [DIM, DIM], BF16, isOutput=False)
    y_d = nc.declare_dram_parameter("y", [BPC, L, DIM], F32, isOutput=True)

    with tile.TileContext(nc) as tc:
        with (
            tc.tile_pool(name="const", bufs=1) as cpool,
            tc.tile_pool(name="inp", bufs=2) as ipool,
            tc.tile_pool(name="work", bufs=2) as wpool,
            tc.tile_pool(name="head", bufs=2) as hpool,
            tc.tile_pool(name="small", bufs=4) as spool,
            tc.tile_pool(name="psum", bufs=3, space="PSUM") as pspool,   # 2-bank slots
            tc.tile_pool(name="psumc", bufs=2, space="PSUM") as pcpool,  # 1-bank slots
        ):
            # ---- constants (loaded once) ----
            wqkvT_sb = cpool.tile([128, 2, 3 * DIM], BF16)  # [p, dchunk, o]
            nc.sync.dma_start(
                out=wqkvT_sb[:, :, :],
                in_=wqkvT_d[:, :].rearrange("(c p) o -> p c o", p=128),
            )
            wprojT_sb = cpool.tile([64, NH, DIM], BF16)  # per head on 64 parts
            nc.sync.dma_start(
                out=wprojT_sb[:, :, :],
                in_=wprojT_d[:, :].rearrange("(h p) o -> p h o", p=64),
            )
            ident_sb = cpool.tile([128, 128], BF16)
            make_identity(nc, ident_sb[:, :])
            ones_src = cpool.tile([128, 16], F32)
            nc.vector.memset(ones_src[:, :], 1.0)
            # dependency-free warm-up activation at kernel start: hoists the
            # ~2.7us exp_and_others ACT_TABLE_LOAD into the initial DMA ramp
            # instead of blocking the first real exp mid-stream
            act_warm = cpool.tile([1, 8], F32)
            nc.scalar.activation(act_warm[:, :], ones_src[0:1, 0:8], AF.Exp)
            # PE HAM warm-up: dependency-free matmuls during the initial DMA
            # ramp lift the PE clock gate to 8/8 (2.4 GHz) before the first
            # real matmuls. Off the critical path; sink read defeats DCE.
            warm_ps = pcpool.tile([128, 128], F32, tag="cnt")
            for _w in range(48):
                nc.tensor.matmul(
                    warm_ps[:, :], lhsT=ident_sb[:, :], rhs=ident_sb[:, :],
                    start=True, stop=True,
                )
            warm_sink = cpool.tile([1, 8], F32)
            nc.scalar.copy(warm_sink[:, :], warm_ps[0:1, 0:8])

            for b in range(BPC):
                # ---------- load ----------
                xT_sb = ipool.tile([128, 2, L], BF16)  # x^T: [p, dchunk, l]
                nc.sync.dma_start(
                    out=xT_sb[:, :, :],
                    in_=xT_d[b].rearrange("(c p) l -> p c l", p=128),
                )
                a8_sb = ipool.tile([128, 4, L], FP8)
                nc.sync.dma_start(
                    out=a8_sb[:, :, :],
                    in_=a8_d[b].rearrange("(c p) j -> p c j", p=128),
                )
                a8T_sb = ipool.tile([128, 4, L], FP8)
                nc.sync.dma_start(
                    out=a8T_sb[:, :, :],
                    in_=a8T_d[b].rearrange("(c p) j -> p c j", p=128),
                )
                mh0_sb = ipool.tile([128, 4, L], BF16)
                nc.sync.dma_start(
                    out=mh0_sb[:, :, :],
                    in_=mh0_d[b].rearrange("(c p) j -> p c j", p=128),
                )
                mh1_sb = ipool.tile([128, 4, L], BF16)
                nc.sync.dma_start(
                    out=mh1_sb[:, :, :],
                    in_=mh1_d[b].rearrange("(c p) j -> p c j", p=128),
                )

                # ---------- QK^T = w_qk @ x^T : [512(o), 512(l)] ----------
                # chunks 0..1 = Q^T (heads 0,1 | 2,3 by 64 rows), 2..3 = K^T
                qkt_sb = wpool.tile([128, 4, L], BF16)
                for op in range(2):  # pairs of output chunks
                    ps = pspool.tile([128, 2, L], F32, tag="ps")
                    for i in range(2):
                        oc = op * 2 + i
                        for c in range(2):
                            nc.tensor.matmul(
                                ps[:, i, :],
                                lhsT=wqkvT_sb[:, c, oc * 128:(oc + 1) * 128],
                                rhs=xT_sb[:, c, :],
                                start=(c == 0),
                                stop=(c == 1),
                            )
                    nc.scalar.copy(qkt_sb[:, op * 2:op * 2 + 2, :], ps[:, :, :])

                # ---------- V (natural layout) + ones column ----------
                v_sb = wpool.tile([128, 4, NH, HD + 1], BF16)
                nc.gpsimd.memset(v_sb[:, :, :, HD:HD + 1], 1.0)
                for lp in range(2):  # pairs of l-chunks
                    psv = pcpool.tile([128, 2, NH * HD], F32, tag="cnt")
                    # one accumulation group for the whole (single-bank) tile
                    for i in range(2):
                        lc = lp * 2 + i
                        for c in range(2):
                            nc.tensor.matmul(
                                psv[:, i, :],
                                lhsT=xT_sb[:, c, lc * 128:(lc + 1) * 128],
                                rhs=wqkvT_sb[:, c, 2 * DIM:3 * DIM],
                                start=(i == 0 and c == 0),
                                stop=(i == 1 and c == 1),
                                skip_group_check=True,
                            )
                    nc.vector.tensor_copy(
                        v_sb[:, lp * 2:lp * 2 + 2, :, 0:HD],
                        psv[:, :, :].rearrange("p i (h d) -> p i h d", h=NH),
                    )

                # ---------- attention ----------
                outTn_sb = wpool.tile([64, NH, L], BF16)  # normalized out^T

                def softmax_pv(h, pt_pair_fn):
                    """pt_pair_fn(kp, pss2, pt_sb): fill pt_sb[:, 2kp:2kp+2, :]
                    from the 2-chunk score psum pss2 [128, 2, L]."""
                    hp = slice((h % 2) * 64, (h % 2) * 64 + 64)
                    qc = h // 2
                    kc_ = 2 + h // 2
                    pt_sb = hpool.tile([128, 4, L], BF16, tag="pt")
                    for kp in range(2):
                        pss2 = pspool.tile([128, 2, L], F32, tag="ps")
                        for i in range(2):
                            kc = kp * 2 + i
                            nc.tensor.matmul(
                                pss2[:, i, :],
                                lhsT=qkt_sb[hp, kc_, kc * 128:(kc + 1) * 128],
                                rhs=qkt_sb[hp, qc, :],
                                start=True,
                                stop=True,
                            )
                        pt_pair_fn(kp, pss2, pt_sb)
                    # [V|1]^T @ P^T: rows 0..63 = out^T, row 64 = rowsums
                    pv = pcpool.tile([HD + 1, L], F32, tag="cnt")
                    for kc in range(4):
                        nc.tensor.matmul(
                            pv[:, :],
                            lhsT=v_sb[:, kc, h, :],
                            rhs=pt_sb[:, kc, :],
                            start=(kc == 0),
                            stop=(kc == 3),
                        )
                    inv_t = spool.tile([65, L], F32, tag="inv")
                    with nc.allow_low_precision(reason="f32 rowsum reciprocal"):
                        nc.vector.reciprocal(inv_t[64:65, :], pv[HD:HD + 1, :])
                    bc_sb = spool.tile([HD, L], F32, tag="bc")
                    nc.gpsimd.partition_broadcast(bc_sb[:, :], inv_t[64:65, :])
                    nc.vector.tensor_mul(
                        outTn_sb[:, h, :], pv[0:HD, :], bc_sb[:, :]
                    )

                # heads 2/3: count matmuls fused into the softmax mask
                def make_pair23(srcb):
                    def pair23(kp, pss2, pt_sb):
                        cnt = pspool.tile([128, 2, L], F32, tag="ps")
                        for i in range(2):
                            kc = kp * 2 + i
                            for kk in (0, 2):
                                nc.tensor.matmul(
                                    cnt[:, i, :],
                                    lhsT=srcb[:, kk:kk + 2, kc * 128:(kc + 1) * 128],
                                    rhs=srcb[:, kk:kk + 2, :],
                                    start=(kk == 0),
                                    stop=False,
                                    perf_mode=DR,
                                )
                            nc.tensor.matmul(
                                cnt[:, i, kc * 128:(kc + 1) * 128],
                                lhsT=ident_sb[:, :],
                                rhs=ident_sb[:, :],
                                start=False,
                                stop=True,
                                skip_group_check=True,
                            )
                        ex = spool.tile([128, 2, L], BF16, tag="ex")
                        nc.scalar.activation(
                            ex[:, :, :], pss2[:, :, :], AF.Exp, scale=1.0 / SCALE
                        )
                        # P^T = (count >= 0.5) * exp
                        nc.vector.scalar_tensor_tensor(
                            pt_sb[:, kp * 2:kp * 2 + 2, :],
                            in0=cnt[:, :, :],
                            scalar=0.5,
                            in1=ex[:, :, :],
                            op0=OP.is_ge,
                            op1=OP.mult,
                        )
                    return pair23

                # heads 0/1: host-prepped (a|I)-masks, multiply on GPSIMD
                def make_pair01(mask):
                    def pair01(kp, pss2, pt_sb):
                        ex = spool.tile([128, 2, L], BF16, tag="ex")
                        nc.scalar.activation(
                            ex[:, :, :], pss2[:, :, :], AF.Exp, scale=1.0 / SCALE
                        )
                        nc.gpsimd.tensor_mul(
                            pt_sb[:, kp * 2:kp * 2 + 2, :],
                            ex[:, :, :],
                            mask[:, kp * 2:kp * 2 + 2, :],
                        )
                    return pair01

                softmax_pv(2, make_pair23(a8_sb))
                softmax_pv(0, make_pair01(mh0_sb))
                softmax_pv(3, make_pair23(a8T_sb))
                softmax_pv(1, make_pair01(mh1_sb))

                # ---------- output projection ----------
                y_sb = wpool.tile([128, 4, DIM], F32)
                for lp in range(2):
                    psy = pcpool.tile([128, 2, DIM], F32, tag="cnt")
                    for i in range(2):
                        lc = lp * 2 + i
                        for h in range(NH):
                            nc.tensor.matmul(
                                psy[:, i, :],
                                lhsT=outTn_sb[:, h, lc * 128:(lc + 1) * 128],
                                rhs=wprojT_sb[:, h, :],
                                start=(i == 0 and h == 0),
                                stop=(i == 1 and h == NH - 1),
                                skip_group_check=True,
                            )
                    nc.scalar.copy(y_sb[:, lp * 2:lp * 2 + 2, :], psy[:, :, :])
                nc.sync.dma_start(
                    out=y_d[b].rearrange("(c p) o -> p c o", p=128),
                    in_=y_sb[:, :, :],
                )
    nc.compile()
    return nc


_CACHED = {}


def _get_nc():
    if "nc" not in _CACHED:
        _CACHED["nc"] = build_nc()
    return _CACHED["nc"]


def shard_inputs(inputs):
    x = np.asarray(inputs["x"], dtype=np.float32)
    adj = np.asarray(inputs["adj"])
    w_qkv = np.asarray(inputs["w_qkv"], dtype=np.float32)
    w_proj = np.asarray(inputs["w_proj"], dtype=np.float32)

    bf16 = ml_dtypes.bfloat16
    fp8 = ml_dtypes.float8_e4m3fn

    xT = np.ascontiguousarray(x.transpose(0, 2, 1)).astype(bf16)  # [B, DIM, L]
    a = ((adj == 1) | (adj >= 9))                                 # [B, L, L] bool
    aT = a.transpose(0, 2, 1)
    eye = np.eye(L, dtype=bool)
    a8 = a.astype(fp8)
    a8T = np.ascontiguousarray(aT).astype(fp8)
    mh0 = (aT | eye).astype(bf16)                                 # head-0 mask^T
    mh1 = (a | eye).astype(bf16)                                  # head-1 mask^T
    wqkvT = np.ascontiguousarray(w_qkv.T).astype(bf16)            # [DIM, 3*DIM]
    wprojT = np.ascontiguousarray(w_proj.T).astype(bf16)          # [DIM, DIM]

    in_maps = []
    for c in range(NCORES):
        sl = slice(c * BPC, (c + 1) * BPC)
        in_maps.append(
            {
                "xT": xT[sl],
                "a8": a8[sl],
                "a8T": a8T[sl],
                "mh0": np.ascontiguousarray(mh0[sl]),
                "mh1": mh1[sl],
                "wqkvT": wqkvT,
                "wprojT": wprojT,
            }
        )
    return in_maps


def kernel(x, adj, w_qkv, w_proj, _want_results_obj=False, **run_kwargs):
    in_maps = shard_inputs(
        {"x": x, "adj": adj, "w_qkv": w_qkv, "w_proj": w_proj}
    )
    nc = _get_nc()
    res = run_bass_kernel_spmd(nc, in_maps, list(range(NCORES)), **run_kwargs)
    y = np.concatenate([res.results[c]["y"] for c in range(NCORES)], axis=0)
    if _want_results_obj:
        return y, res
    return y


# revision 13
# speedup vs baseline: 1.1536x; 1.1536x over previous
"""Trainium2 Bass kernel for masked multi-head attention with adjacency-derived
sparse masks (nn_MultiHeadAttention_4922032521398).

Reference (per batch of 32, L=512, DIM=256, 4 heads x 64):
    qkv = x @ w_qkv.T ; q,k,v per head
    score = q @ k.T / sqrt(64)
    a   = binarize(adj): 1 where adj==1 or adj>=9 else 0
    pe  = stack([a, aT, aT@a, a@aT]) + I   (per-head masks, !=0 -> keep)
    out = softmax(where(pe==0, -inf, score)) @ v ; y = out @ w_proj.T

Strategy (data-parallel over batch across 8 cores, 4 batches each):
  - Scores built transposed: S^T[k,q] so attention@V and the projection
    contract without any on-device transposes.  P^T = exp(S^T/8)*mask^T;
    scores are small (|s|<~2) so exp needs no max-subtraction, and the 0/1
    mask multiply equals -inf masking exactly.
  - Host precomputes the adjacency binarization: pure 0/1 bins go up as
    fp8 (exact) for the head-2/3 count matmuls; (a|I)/(aT|I) go up as bf16
    and are used directly as the head-0/1 masks.
  - Heads 2/3: adjacency counts (aT@a / a@aT as fp8 DoubleRow matmuls on
    exact 0/1 values, fp32 PSUM accumulate => exact counts) stay in PSUM
    and fuse into the softmax as P^T=(count>=0.5)*exp(S^T/8) in one DVE
    scalar_tensor_tensor op; the +I lands via a 128-col identity matmul
    that also closes the accumulation group.
  - Row sums via a ones-column appended to V (PV matmul row 64); applied
    as DVE reciprocal -> GPSIMD partition_broadcast -> DVE multiply.
  - Software-pipelined head schedule: each head's PV/normalization is
    emitted after the NEXT head's score/count matmuls so the PE never
    sits behind the ACT exp / DVE mask chain; the next batch's QK/V is
    emitted before the last head's PV for the same reason.
  - Q/K copies out of PSUM are paired (Q01+K01 first) so the first head
    pair's scores unblock after one copy.
  - Output DMA runs on the Pool DGE queue so it never head-of-line
    blocks the SP queue that streams the next batch's inputs.
  - x / weights / P / V in bf16; all matmuls full-rate.
"""

import os
import sys

os.environ.setdefault("JAX_PLATFORMS", "axon,cpu")

for _p in ("/opt/trn_rl_repo",):
    if _p not in sys.path:
        sys.path.append(_p)

import numpy as np
import ml_dtypes

import concourse.bass as bass
import concourse.mybir as mybir
import concourse.tile as tile
from concourse import bacc
from concourse.bass_utils import run_bass_kernel_spmd
from concourse.masks import make_identity

B, L, DIM, NH = 32, 512, 256, 4
HD = DIM // NH  # 64
SCALE = float(np.sqrt(HD))
NCORES = 8
BPC = B // NCORES  # batches per core

F32 = mybir.dt.float32
BF16 = mybir.dt.bfloat16
FP8 = mybir.dt.float8e4
AF = mybir.ActivationFunctionType
OP = mybir.AluOpType
DR = mybir.MatmulPerfMode.DoubleRow

HEAD_ORDER = (0, 2, 1, 3)


def build_nc():
    nc = bacc.Bacc("TRN2", target_bir_lowering=False)
    xT_d = nc.declare_dram_parameter("xT", [BPC, DIM, L], BF16, isOutput=False)
    # packed pure bins: [:, 0] = a, [:, 1] = aT (fp8, exact 0/1)
    a8p_d = nc.declare_dram_parameter("a8p", [BPC, 2, L, L], FP8, isOutput=False)
    # packed head-0/1 masks: [:, 0] = (aT|I), [:, 1] = (a|I)
    mhp_d = nc.declare_dram_parameter("mhp", [BPC, 2, L, L], BF16, isOutput=False)
    wqkvT_d = nc.declare_dram_parameter("wqkvT", [DIM, 3 * DIM], BF16, isOutput=False)
    wprojT_d = nc.declare_dram_parameter("wprojT", [DIM, DIM], BF16, isOutput=False)
    y_d = nc.declare_dram_parameter("y", [BPC, L, DIM], F32, isOutput=True)

    with tile.TileContext(nc) as tc:
        with (
            tc.tile_pool(name="const", bufs=1) as cpool,
            tc.tile_pool(name="inp", bufs=2) as ipool,
            tc.tile_pool(name="work", bufs=2) as wpool,
            tc.tile_pool(name="head", bufs=2) as hpool,
            tc.tile_pool(name="small", bufs=4) as spool,
            tc.tile_pool(name="psum", bufs=2, space="PSUM") as pspool,   # 2-bank slots
            tc.tile_pool(name="psumc", bufs=4, space="PSUM") as pcpool,  # 1-bank slots
        ):
            # ---- constants (loaded once) ----
            wqkvT_sb = cpool.tile([128, 2, 3 * DIM], BF16)  # [p, dchunk, o]
            nc.sync.dma_start(
                out=wqkvT_sb[:, :, :],
                in_=wqkvT_d[:, :].rearrange("(c p) o -> p c o", p=128),
            )
            wprojT_sb = cpool.tile([64, NH, DIM], BF16)  # per head on 64 parts
            nc.sync.dma_start(
                out=wprojT_sb[:, :, :],
                in_=wprojT_d[:, :].rearrange("(h p) o -> p h o", p=64),
            )
            ident_sb = cpool.tile([128, 128], BF16)
            make_identity(nc, ident_sb[:, :])
            ones_src = cpool.tile([128, 16], F32)
            nc.vector.memset(ones_src[:, :], 1.0)
            # dependency-free warm-up activation at kernel start: hoists the
            # exp ACT_TABLE_LOAD into the initial DMA ramp
            act_warm = cpool.tile([1, 8], F32)
            nc.scalar.activation(act_warm[:, :], ones_src[0:1, 0:8], AF.Exp)
            # PE warm-up: dependency-free matmuls during the initial DMA ramp
            # lift the PE clock to 2.4 GHz before the first real matmuls.
            warm_ps = pcpool.tile([128, 128], F32, tag="cnt")
            for _w in range(32):
                nc.tensor.matmul(
                    warm_ps[:, :], lhsT=ident_sb[:, :], rhs=ident_sb[:, :],
                    start=True, stop=True,
                )
            warm_sink = cpool.tile([1, 8], F32)
            nc.scalar.copy(warm_sink[:, :], warm_ps[0:1, 0:8])

            def pre(b):
                """Loads + QK^T + V for batch b."""
                st = {}
                xT_sb = ipool.tile([128, 2, L], BF16, tag="xT")
                st["xT"] = xT_sb
                nc.sync.dma_start(
                    out=xT_sb[:, :, :],
                    in_=xT_d[b].rearrange("(c p) l -> p c l", p=128),
                )
                a8p_sb = ipool.tile([128, 2, 4, L], FP8, tag="a8p")
                st["a8p"] = a8p_sb
                nc.sync.dma_start(
                    out=a8p_sb[:, :, :, :],
                    in_=a8p_d[b].rearrange("t (c p) j -> p t c j", p=128),
                )
                mhp_sb = ipool.tile([128, 2, 4, L], BF16, tag="mhp")
                st["mhp"] = mhp_sb
                nc.sync.dma_start(
                    out=mhp_sb[:, :, :, :],
                    in_=mhp_d[b].rearrange("t (c p) j -> p t c j", p=128),
                )

                # QK^T = w_qk @ x^T, grouped so one PSUM->SBUF copy delivers
                # (Q01, K01) [head pair 0] and the next (Q23, K23).
                # qkt[p, 0=Q/1=K, hpair, l]
                qkt_sb = wpool.tile([128, 2, 2, L], BF16, tag="qkt")
                st["qkt"] = qkt_sb
                for hp_ in range(2):  # head pair
                    ps = pspool.tile([128, 2, L], F32, tag="ps")
                    for i, oc in enumerate((hp_, 2 + hp_)):  # Q chunk, K chunk
                        for c in range(2):
                            nc.tensor.matmul(
                                ps[:, i, :],
                                lhsT=wqkvT_sb[:, c, oc * 128:(oc + 1) * 128],
                                rhs=xT_sb[:, c, :],
                                start=(c == 0),
                                stop=(c == 1),
                            )
                    nc.scalar.copy(qkt_sb[:, :, hp_, :], ps[:, :, :])

                # V (natural layout) + ones column
                v_sb = wpool.tile([128, 4, NH, HD + 1], BF16, tag="v")
                st["v"] = v_sb
                nc.gpsimd.memset(v_sb[:, :, :, HD:HD + 1], 1.0)
                for lp in range(2):
                    psv = pcpool.tile([128, 2, NH * HD], F32, tag="cnt")
                    for i in range(2):
                        lc = lp * 2 + i
                        for c in range(2):
                            nc.tensor.matmul(
                                psv[:, i, :],
                                lhsT=xT_sb[:, c, lc * 128:(lc + 1) * 128],
                                rhs=wqkvT_sb[:, c, 2 * DIM:3 * DIM],
                                start=(i == 0 and c == 0),
                                stop=(i == 1 and c == 1),
                                skip_group_check=True,
                            )
                    nc.vector.tensor_copy(
                        v_sb[:, lp * 2:lp * 2 + 2, :, 0:HD],
                        psv[:, :, :].rearrange("p i (h d) -> p i h d", h=NH),
                    )
                outTn_sb = wpool.tile([64, NH, L], BF16, tag="outTn", name="outTn")
                st["outTn"] = outTn_sb
                return st

            def head_front(st, h):
                """Scores (+counts) -> exp -> mask => pt tile for head h."""
                qkt_sb = st["qkt"]
                hp = slice((h % 2) * 64, (h % 2) * 64 + 64)
                hpair = h // 2
                pt_sb = hpool.tile([128, 4, L], BF16, tag="pt")
                st[("pt", h)] = pt_sb
                if h >= 2:
                    srcb = st["a8p"][:, h - 2]
                else:
                    srcb = None
                    mask = st["mhp"][:, h]
                for kp in range(2):
                    pss2 = pspool.tile([128, 2, L], F32, tag="ps")
                    for i in range(2):
                        kc = kp * 2 + i
                        nc.tensor.matmul(
                            pss2[:, i, :],
                            lhsT=qkt_sb[hp, 1, hpair, kc * 128:(kc + 1) * 128],
                            rhs=qkt_sb[hp, 0, hpair, :],
                            start=True,
                            stop=True,
                        )
                    if h >= 2:
                        cnt = pspool.tile([128, 2, L], F32, tag="ps")
                        for i in range(2):
                            kc = kp * 2 + i
                            for kk in (0, 2):
                                nc.tensor.matmul(
                                    cnt[:, i, :],
                                    lhsT=srcb[:, kk:kk + 2, kc * 128:(kc + 1) * 128],
                                    rhs=srcb[:, kk:kk + 2, :],
                                    start=(kk == 0),
                                    stop=False,
                                    perf_mode=DR,
                                )
                            nc.tensor.matmul(
                                cnt[:, i, kc * 128:(kc + 1) * 128],
                                lhsT=ident_sb[:, :],
                                rhs=ident_sb[:, :],
                                start=False,
                                stop=True,
                                skip_group_check=True,
                            )
                    ex = spool.tile([128, 2, L], BF16, tag="ex")
                    nc.scalar.activation(
                        ex[:, :, :], pss2[:, :, :], AF.Exp, scale=1.0 / SCALE
                    )
                    if h >= 2:
                        # P^T = (count >= 0.5) * exp
                        nc.vector.scalar_tensor_tensor(
                            pt_sb[:, kp * 2:kp * 2 + 2, :],
                            in0=cnt[:, :, :],
                            scalar=0.5,
                            in1=ex[:, :, :],
                            op0=OP.is_ge,
                            op1=OP.mult,
                        )
                    else:
                        nc.vector.tensor_mul(
                            pt_sb[:, kp * 2:kp * 2 + 2, :],
                            ex[:, :, :],
                            mask[:, kp * 2:kp * 2 + 2, :],
                        )

            def head_back(st, h):
                """PV + normalization => outTn[:, h, :]."""
                v_sb = st["v"]
                pt_sb = st.pop(("pt", h))
                # [V|1]^T @ P^T: rows 0..63 = out^T, row 64 = rowsums
                pv = pcpool.tile([HD + 1, L], F32, tag="cnt")
                for kc in range(4):
                    nc.tensor.matmul(
                        pv[:, :],
                        lhsT=v_sb[:, kc, h, :],
                        rhs=pt_sb[:, kc, :],
                        start=(kc == 0),
                        stop=(kc == 3),
                    )
                inv_t = spool.tile([65, L], F32, tag="inv")
                with nc.allow_low_precision(reason="f32 rowsum reciprocal"):
                    nc.vector.reciprocal(inv_t[64:65, :], pv[HD:HD + 1, :])
                bc_sb = spool.tile([HD, L], F32, tag="bc")
                nc.gpsimd.partition_broadcast(bc_sb[:, :], inv_t[64:65, :])
                nc.vector.tensor_mul(
                    st["outTn"][:, h, :], pv[0:HD, :], bc_sb[:, :]
                )

            def proj(b, st):
                outTn_sb = st["outTn"]
                y_sb = wpool.tile([128, 4, DIM], F32, tag="y")
                for lp in range(2):
                    psy = pcpool.tile([128, 2, DIM], F32, tag="cnt")
                    for i in range(2):
                        lc = lp * 2 + i
                        for h in range(NH):
                            nc.tensor.matmul(
                                psy[:, i, :],
                                lhsT=outTn_sb[:, h, lc * 128:(lc + 1) * 128],
                                rhs=wprojT_sb[:, h, :],
                                start=(i == 0 and h == 0),
                                stop=(i == 1 and h == NH - 1),
                                skip_group_check=True,
                            )
                    nc.scalar.copy(y_sb[:, lp * 2:lp * 2 + 2, :], psy[:, :, :])
                # output DMA on the Pool DGE queue: keeps the SP queue free
                # for the next batch's input loads
                nc.gpsimd.dma_start(
                    out=y_d[b].rearrange("(c p) o -> p c o", p=128),
                    in_=y_sb[:, :, :],
                )

            st = pre(0)
            for b in range(BPC):
                head_front(st, HEAD_ORDER[0])
                head_front(st, HEAD_ORDER[1])
                head_back(st, HEAD_ORDER[0])
                head_front(st, HEAD_ORDER[2])
                head_back(st, HEAD_ORDER[1])
                head_front(st, HEAD_ORDER[3])
                head_back(st, HEAD_ORDER[2])
                nxt = pre(b + 1) if b + 1 < BPC else None
                head_back(st, HEAD_ORDER[3])
                proj(b, st)
                st = nxt
    nc.compile()
    return nc


_CACHED = {}


def _get_nc():
    if "nc" not in _CACHED:
        _CACHED["nc"] = build_nc()
    return _CACHED["nc"]


def shard_inputs(inputs):
    x = np.asarray(inputs["x"], dtype=np.float32)
    adj = np.asarray(inputs["adj"])
    w_qkv = np.asarray(inputs["w_qkv"], dtype=np.float32)
    w_proj = np.asarray(inputs["w_proj"], dtype=np.float32)

    bf16 = ml_dtypes.bfloat16
    fp8 = ml_dtypes.float8_e4m3fn

    xT = np.ascontiguousarray(x.transpose(0, 2, 1)).astype(bf16)  # [B, DIM, L]
    a = ((adj == 1) | (adj >= 9))                                 # [B, L, L] bool
    aT = a.transpose(0, 2, 1)
    eye = np.eye(L, dtype=bool)
    # packed pure bins (fp8) and (·|I)-masks (bf16): [B, 2, L, L]
    a8p = np.stack([a, aT], axis=1).astype(fp8)
    mhp = np.stack([aT | eye, a | eye], axis=1).astype(bf16)
    wqkvT = np.ascontiguousarray(w_qkv.T).astype(bf16)            # [DIM, 3*DIM]
    wprojT = np.ascontiguousarray(w_proj.T).astype(bf16)          # [DIM, DIM]

    in_maps = []
    for c in range(NCORES):
        sl = slice(c * BPC, (c + 1) * BPC)
        in_maps.append(
            {
                "xT": xT[sl],
                "a8p": a8p[sl],
                "mhp": mhp[sl],
                "wqkvT": wqkvT,
                "wprojT": wprojT,
            }
        )
    return in_maps


def kernel(x, adj, w_qkv, w_proj, _want_results_obj=False, **run_kwargs):
    in_maps = shard_inputs(
        {"x": x, "adj": adj, "w_qkv": w_qkv, "w_proj": w_proj}
    )
    nc = _get_nc()
    res = run_bass_kernel_spmd(nc, in_maps, list(range(NCORES)), **run_kwargs)
    y = np.concatenate([res.results[c]["y"] for c in range(NCORES)], axis=0)
    if _want_results_obj:
        return y, res
    return y


# revision 19
# speedup vs baseline: 1.2209x; 1.0583x over previous
"""Trainium2 Bass kernel for masked multi-head attention with adjacency-derived
sparse masks (nn_MultiHeadAttention_4922032521398).

Reference (per batch of 32, L=512, DIM=256, 4 heads x 64):
    qkv = x @ w_qkv.T ; q,k,v per head
    score = q @ k.T / sqrt(64)
    a   = binarize(adj): 1 where adj==1 or adj>=9 else 0
    pe  = stack([a, aT, aT@a, a@aT]) + I   (per-head masks, !=0 -> keep)
    out = softmax(where(pe==0, -inf, score)) @ v ; y = out @ w_proj.T

Strategy (data-parallel over batch across 8 cores, 4 batches each):
  - Scores built transposed: S^T[k,q] so attention@V and the projection
    contract without any on-device transposes.  P^T = exp(S^T/8)*mask^T;
    scores are small (|s|<~2) so exp needs no max-subtraction, and the 0/1
    mask multiply equals -inf masking exactly.
  - Host precomputes the adjacency binarization: pure 0/1 bins go up as
    fp8 (exact) for the head-2/3 count matmuls; (a|I)/(aT|I) go up as bf16
    and are used directly as the head-0/1 masks.
  - Heads 2/3: adjacency counts (aT@a / a@aT as fp8 DoubleRow matmuls on
    exact 0/1 values, fp32 PSUM accumulate => exact counts) stay in PSUM
    and fuse into the softmax as P^T=(count>=0.5)*exp(S^T/8) in one DVE
    scalar_tensor_tensor op; the +I lands via a 128-col identity matmul
    that also closes the accumulation group.
  - Row sums via a ones-column appended to V (PV matmul row 64); applied
    as DVE reciprocal -> GPSIMD partition_broadcast -> DVE multiply.
  - Software-pipelined head schedule: each head's PV/normalization is
    emitted after the NEXT head's score/count matmuls so the PE never
    sits behind the ACT exp / DVE mask chain; the next batch's QK/V is
    emitted before the last head's PV for the same reason.
  - Q/K copies out of PSUM are paired (Q01+K01 first) so the first head
    pair's scores unblock after one copy.
  - Output DMA runs on the Pool DGE queue so it never head-of-line
    blocks the SP queue that streams the next batch's inputs.
  - x / weights / P / V in bf16; all matmuls full-rate.
"""

import os
import sys

os.environ.setdefault("JAX_PLATFORMS", "axon,cpu")

for _p in ("/opt/trn_rl_repo",):
    if _p not in sys.path:
        sys.path.append(_p)

import numpy as np
import ml_dtypes

import concourse.bass as bass
import concourse.mybir as mybir
import concourse.tile as tile
from concourse import bacc
from concourse.bass_utils import run_bass_kernel_spmd
from concourse.masks import make_identity

B, L, DIM, NH = 32, 512, 256, 4
HD = DIM // NH  # 64
SCALE = float(np.sqrt(HD))
NCORES = 8
BPC = B // NCORES  # batches per core

F32 = mybir.dt.float32
BF16 = mybir.dt.bfloat16
FP8 = mybir.dt.float8e4
AF = mybir.ActivationFunctionType
OP = mybir.AluOpType
DR = mybir.MatmulPerfMode.DoubleRow

HEAD_ORDER = (0, 2, 1, 3)


def build_nc():
    nc = bacc.Bacc("TRN2", target_bir_lowering=False)
    xT_d = nc.declare_dram_parameter("xT", [BPC, DIM, L], BF16, isOutput=False)
    # packed pure bins: [:, 0] = a, [:, 1] = aT (fp8, exact 0/1)
    a8p_d = nc.declare_dram_parameter("a8p", [BPC, 2, L, L], FP8, isOutput=False)
    # packed head-0/1 masks: [:, 0] = (aT|I), [:, 1] = (a|I)
    mhp_d = nc.declare_dram_parameter("mhp", [BPC, 2, L, L], BF16, isOutput=False)
    wqkvT_d = nc.declare_dram_parameter("wqkvT", [DIM, 3 * DIM], BF16, isOutput=False)
    wprojT_d = nc.declare_dram_parameter("wprojT", [DIM, DIM], BF16, isOutput=False)
    y_d = nc.declare_dram_parameter("y", [BPC, L, DIM], F32, isOutput=True)

    with tile.TileContext(nc) as tc:
        with (
            tc.tile_pool(name="const", bufs=1) as cpool,
            tc.tile_pool(name="inp", bufs=2) as ipool,
            tc.tile_pool(name="work", bufs=2) as wpool,
            tc.tile_pool(name="head", bufs=2) as hpool,
            tc.tile_pool(name="small", bufs=4) as spool,
            tc.tile_pool(name="psum", bufs=2, space="PSUM") as pspool,   # 2-bank slots
            tc.tile_pool(name="psumc", bufs=4, space="PSUM") as pcpool,  # 1-bank slots
        ):
            # ---- constants (loaded once) ----
            wqkvT_sb = cpool.tile([128, 2, 3 * DIM], BF16)  # [p, dchunk, o]
            nc.sync.dma_start(
                out=wqkvT_sb[:, :, :],
                in_=wqkvT_d[:, :].rearrange("(c p) o -> p c o", p=128),
            )
            wprojT_sb = cpool.tile([64, NH, DIM], BF16)  # per head on 64 parts
            nc.sync.dma_start(
                out=wprojT_sb[:, :, :],
                in_=wprojT_d[:, :].rearrange("(h p) o -> p h o", p=64),
            )
            ident_sb = cpool.tile([128, 128], BF16)
            make_identity(nc, ident_sb[:, :])
            ones_src = cpool.tile([128, 16], F32)
            nc.vector.memset(ones_src[:, :], 1.0)
            # dependency-free warm-up activation at kernel start: hoists the
            # exp ACT_TABLE_LOAD into the initial DMA ramp
            act_warm = cpool.tile([1, 8], F32)
            nc.scalar.activation(act_warm[:, :], ones_src[0:1, 0:8], AF.Exp)
            # PE warm-up: dependency-free matmuls during the initial DMA ramp
            # lift the PE clock to 2.4 GHz before the first real matmuls.
            warm_ps = pcpool.tile([128, 128], F32, tag="cnt")
            for _w in range(32):
                nc.tensor.matmul(
                    warm_ps[:, :], lhsT=ident_sb[:, :], rhs=ident_sb[:, :],
                    start=True, stop=True,
                )
            warm_sink = cpool.tile([1, 8], F32)
            nc.scalar.copy(warm_sink[:, :], warm_ps[0:1, 0:8])

            def pre(b):
                """Loads + QK^T + V for batch b."""
                st = {}
                xT_sb = ipool.tile([128, 2, L], BF16, tag="xT")
                st["xT"] = xT_sb
                nc.sync.dma_start(
                    out=xT_sb[:, :, :],
                    in_=xT_d[b].rearrange("(c p) l -> p c l", p=128),
                )
                mhp_sb = ipool.tile([128, 2, 4, L], BF16, tag="mhp")
                st["mhp"] = mhp_sb
                nc.sync.dma_start(
                    out=mhp_sb[:, :, :, :],
                    in_=mhp_d[b].rearrange("t (c p) j -> p t c j", p=128),
                )
                a8p_sb = ipool.tile([128, 2, 4, L], FP8, tag="a8p")
                st["a8p"] = a8p_sb
                nc.sync.dma_start(
                    out=a8p_sb[:, :, :, :],
                    in_=a8p_d[b].rearrange("t (c p) j -> p t c j", p=128),
                )

                # QK^T = w_qk @ x^T, grouped so one PSUM->SBUF copy delivers
                # (Q01, K01) [head pair 0] and the next (Q23, K23).
                # qkt[p, 0=Q/1=K, hpair, l]
                qkt_sb = wpool.tile([128, 2, 2, L], BF16, tag="qkt")
                st["qkt"] = qkt_sb
                for hp_ in range(2):  # head pair
                    ps = pspool.tile([128, 2, L], F32, tag="ps")
                    for i, oc in enumerate((hp_, 2 + hp_)):  # Q chunk, K chunk
                        for c in range(2):
                            nc.tensor.matmul(
                                ps[:, i, :],
                                lhsT=wqkvT_sb[:, c, oc * 128:(oc + 1) * 128],
                                rhs=xT_sb[:, c, :],
                                start=(c == 0),
                                stop=(c == 1),
                            )
                    nc.scalar.copy(qkt_sb[:, :, hp_, :], ps[:, :, :])

                # V (natural layout) + ones column
                v_sb = wpool.tile([128, 4, NH, HD + 1], BF16, tag="v")
                st["v"] = v_sb
                nc.gpsimd.memset(v_sb[:, :, :, HD:HD + 1], 1.0)
                for lp in range(2):
                    psv = pcpool.tile([128, 2, NH * HD], F32, tag="cnt")
                    for i in range(2):
                        lc = lp * 2 + i
                        for c in range(2):
                            nc.tensor.matmul(
                                psv[:, i, :],
                                lhsT=xT_sb[:, c, lc * 128:(lc + 1) * 128],
                                rhs=wqkvT_sb[:, c, 2 * DIM:3 * DIM],
                                start=(i == 0 and c == 0),
                                stop=(i == 1 and c == 1),
                                skip_group_check=True,
                            )
                    nc.vector.tensor_copy(
                        v_sb[:, lp * 2:lp * 2 + 2, :, 0:HD],
                        psv[:, :, :].rearrange("p i (h d) -> p i h d", h=NH),
                    )
                outTn_sb = wpool.tile([64, NH, L], BF16, tag="outTn", name="outTn")
                st["outTn"] = outTn_sb
                return st

            def head_front(st, h):
                """Scores (+counts) -> exp -> mask => pt tile for head h."""
                qkt_sb = st["qkt"]
                hp = slice((h % 2) * 64, (h % 2) * 64 + 64)
                hpair = h // 2
                pt_sb = hpool.tile([128, 4, L], BF16, tag="pt")
                st[("pt", h)] = pt_sb
                if h >= 2:
                    srcb = st["a8p"][:, h - 2]
                else:
                    srcb = None
                    mask = st["mhp"][:, h]
                for kp in range(2):
                    pss2 = pspool.tile([128, 2, L], F32, tag="ps")
                    for i in range(2):
                        kc = kp * 2 + i
                        nc.tensor.matmul(
                            pss2[:, i, :],
                            lhsT=qkt_sb[hp, 1, hpair, kc * 128:(kc + 1) * 128],
                            rhs=qkt_sb[hp, 0, hpair, :],
                            start=True,
                            stop=True,
                        )
                    if h >= 2:
                        cnt = pspool.tile([128, 2, L], F32, tag="ps")
                        for i in range(2):
                            kc = kp * 2 + i
                            for kk in (0, 2):
                                nc.tensor.matmul(
                                    cnt[:, i, :],
                                    lhsT=srcb[:, kk:kk + 2, kc * 128:(kc + 1) * 128],
                                    rhs=srcb[:, kk:kk + 2, :],
                                    start=(kk == 0),
                                    stop=False,
                                    perf_mode=DR,
                                )
                            nc.tensor.matmul(
                                cnt[:, i, kc * 128:(kc + 1) * 128],
                                lhsT=ident_sb[:, :],
                                rhs=ident_sb[:, :],
                                start=False,
                                stop=True,
                                skip_group_check=True,
                            )
                    ex = spool.tile([128, 2, L], BF16, tag="ex")
                    nc.scalar.activation(
                        ex[:, :, :], pss2[:, :, :], AF.Exp, scale=1.0 / SCALE
                    )
                    if h >= 2:
                        # P^T = (count >= 0.5) * exp
                        nc.vector.scalar_tensor_tensor(
                            pt_sb[:, kp * 2:kp * 2 + 2, :],
                            in0=cnt[:, :, :],
                            scalar=0.5,
                            in1=ex[:, :, :],
                            op0=OP.is_ge,
                            op1=OP.mult,
                        )
                    else:
                        nc.vector.tensor_mul(
                            pt_sb[:, kp * 2:kp * 2 + 2, :],
                            ex[:, :, :],
                            mask[:, kp * 2:kp * 2 + 2, :],
                        )

            def head_back(st, h):
                """PV + normalization => outTn[:, h, :]."""
                v_sb = st["v"]
                pt_sb = st.pop(("pt", h))
                # [V|1]^T @ P^T: rows 0..63 = out^T, row 64 = rowsums
                pv = pcpool.tile([HD + 1, L], F32, tag="cnt")
                for kc in range(4):
                    nc.tensor.matmul(
                        pv[:, :],
                        lhsT=v_sb[:, kc, h, :],
                        rhs=pt_sb[:, kc, :],
                        start=(kc == 0),
                        stop=(kc == 3),
                    )
                inv_t = spool.tile([65, L], F32, tag="inv")
                with nc.allow_low_precision(reason="f32 rowsum reciprocal"):
                    nc.vector.reciprocal(inv_t[64:65, :], pv[HD:HD + 1, :])
                bc_sb = spool.tile([HD, L], F32, tag="bc")
                nc.gpsimd.partition_broadcast(bc_sb[:, :], inv_t[64:65, :])
                nc.vector.tensor_mul(
                    st["outTn"][:, h, :], pv[0:HD, :], bc_sb[:, :]
                )

            def proj(b, st):
                outTn_sb = st["outTn"]
                y_sb = wpool.tile([128, 4, DIM], F32, tag="y")
                for lp in range(2):
                    psy = pcpool.tile([128, 2, DIM], F32, tag="cnt")
                    for i in range(2):
                        lc = lp * 2 + i
                        for h in range(NH):
                            nc.tensor.matmul(
                                psy[:, i, :],
                                lhsT=outTn_sb[:, h, lc * 128:(lc + 1) * 128],
                                rhs=wprojT_sb[:, h, :],
                                start=(i == 0 and h == 0),
                                stop=(i == 1 and h == NH - 1),
                                skip_group_check=True,
                            )
                    nc.scalar.copy(y_sb[:, lp * 2:lp * 2 + 2, :], psy[:, :, :])
                    # per-half output DMA on the Pool DGE queue: overlaps the
                    # second half's projection and keeps the SP queue free
                    nc.gpsimd.dma_start(
                        out=y_d[b, lp * 256:(lp + 1) * 256].rearrange(
                            "(c p) o -> p c o", p=128
                        ),
                        in_=y_sb[:, lp * 2:lp * 2 + 2, :],
                    )

            st = pre(0)
            head_front(st, HEAD_ORDER[0])
            for b in range(BPC):
                head_front(st, HEAD_ORDER[1])
                head_back(st, HEAD_ORDER[0])
                head_front(st, HEAD_ORDER[2])
                head_back(st, HEAD_ORDER[1])
                head_front(st, HEAD_ORDER[3])
                head_back(st, HEAD_ORDER[2])
                nxt = pre(b + 1) if b + 1 < BPC else None
                head_back(st, HEAD_ORDER[3])
                if nxt is not None:
                    head_front(nxt, HEAD_ORDER[0])
                proj(b, st)
                st = nxt
    nc.compile()
    return nc


_CACHED = {}


def _get_nc():
    if "nc" not in _CACHED:
        _CACHED["nc"] = build_nc()
    return _CACHED["nc"]


def shard_inputs(inputs):
    x = np.asarray(inputs["x"], dtype=np.float32)
    adj = np.asarray(inputs["adj"])
    w_qkv = np.asarray(inputs["w_qkv"], dtype=np.float32)
    w_proj = np.asarray(inputs["w_proj"], dtype=np.float32)

    bf16 = ml_dtypes.bfloat16
    fp8 = ml_dtypes.float8_e4m3fn

    xT = np.ascontiguousarray(x.transpose(0, 2, 1)).astype(bf16)  # [B, DIM, L]
    a = ((adj == 1) | (adj >= 9))                                 # [B, L, L] bool
    aT = a.transpose(0, 2, 1)
    eye = np.eye(L, dtype=bool)
    # packed pure bins (fp8) and (·|I)-masks (bf16): [B, 2, L, L]
    a8p = np.stack([a, aT], axis=1).astype(fp8)
    mhp = np.stack([aT | eye, a | eye], axis=1).astype(bf16)
    wqkvT = np.ascontiguousarray(w_qkv.T).astype(bf16)            # [DIM, 3*DIM]
    wprojT = np.ascontiguousarray(w_proj.T).astype(bf16)          # [DIM, DIM]

    in_maps = []
    for c in range(NCORES):
        sl = slice(c * BPC, (c + 1) * BPC)
        in_maps.append(
            {
                "xT": xT[sl],
                "a8p": a8p[sl],
                "mhp": mhp[sl],
                "wqkvT": wqkvT,
                "wprojT": wprojT,
            }
        )
    return in_maps


def kernel(x, adj, w_qkv, w_proj, _want_results_obj=False, **run_kwargs):
    in_maps = shard_inputs(
        {"x": x, "adj": adj, "w_qkv": w_qkv, "w_proj": w_proj}
    )
    nc = _get_nc()
    res = run_bass_kernel_spmd(nc, in_maps, list(range(NCORES)), **run_kwargs)
    y = np.concatenate([res.results[c]["y"] for c in range(NCORES)], axis=0)
    if _want_results_obj:
        return y, res
    return y
